# revision 14
# baseline (speedup 1.0000x reference)
"""AttentionPooling (segment softmax pooling) on 8 Trainium2 NeuronCores.

Strategy (data parallel, zero cross-core communication):
  - batch is sorted, so each segment's nodes are contiguous. Host groups
    segments into blocks of K=64 ("groups"), assigns 32 groups (2048 segments)
    to each of the 8 cores, and pads every group's node slice to a fixed PAD
    (multiple of 128*TPC) so the SPMD program has static shapes.
  - Per 128-node tile the device computes scores s = leakyrelu(x @ a) via a
    fused DVE multiply+reduce, w = exp(s) on ACT (no max subtraction needed:
    scores ~ N(0, 128) keep exp(s) well inside fp32 range, and the softmax
    ratio is identical), then builds a w-scaled one-hot selector
    M[node, seg_local] = w * (iota == seg_local) in one fused tensor_scalar,
    and accumulates num/den with a single PE matmul per tile:
        psum[K, D+1] += M.T @ [x | 1]
  - Group epilogue: out = num * reciprocal(den + 1e-16), DMA to DRAM.
Padded rows carry x=0 and a sentinel seg id (200) so they hit no selector
column and contribute nothing.
"""

import numpy as np

N_NODES = 2_000_000
D = 128
NSEG = 16384
NCORES = 8
K = 64                       # segments per group (selector width)
GPC = NSEG // NCORES // K    # 32 groups per core
NEG_SLOPE = 0.2
TPC = 16                     # tiles per DMA chunk (16*128 nodes = 1 MiB of x)

_prog_cache = {}


def _build_program(pad):
    from concourse import bacc, mybir, tile
    import concourse.bass as bass

    f32 = mybir.dt.float32
    f32r = mybir.dt.float32r
    tiles_per_group = pad // 128
    chunks_per_group = pad // (128 * TPC)

    nc = bacc.Bacc(
        "TRN2",
        target_bir_lowering=False,
        debug=False,
        enable_asserts=False,
        num_devices=NCORES,
    )

    xg = nc.dram_tensor("xg", [GPC * pad, D], f32, kind="ExternalInput")
    # bl pre-tiled on host: [group, chunk, partition(node%128), tile]
    bl = nc.dram_tensor("bl", [GPC, chunks_per_group, 128, TPC], f32, kind="ExternalInput")
    arep = nc.dram_tensor("arep", [128, TPC, D], f32, kind="ExternalInput")
    iota_in = nc.dram_tensor("iota_in", [128, K], f32, kind="ExternalInput")
    ones_in = nc.dram_tensor("ones_in", [128, TPC, 4], f32, kind="ExternalInput")
    out = nc.dram_tensor("out", [GPC * K, D], f32, kind="ExternalOutput")

    with tile.TileContext(nc) as tc:
        with (
            tc.tile_pool(name="const", bufs=1) as constp,
            tc.tile_pool(name="xch", bufs=4) as xpool,
            tc.tile_pool(name="blp", bufs=4) as blpool,
            tc.tile_pool(name="sc", bufs=4) as scpool,
            tc.tile_pool(name="scr", bufs=2) as scrpool,
            tc.tile_pool(name="xr", bufs=3) as xrpool,
            tc.tile_pool(name="m", bufs=4) as mpool,
            tc.tile_pool(name="ep", bufs=2) as eppool,
            tc.tile_pool(name="ps", bufs=2, space="PSUM") as psump,
        ):
            a_sb = constp.tile([128, TPC, D], f32, tag="a")
            nc.sync.dma_start(out=a_sb[:, :, :], in_=arep[:, :, :])
            iota_sb = constp.tile([128, K], f32, tag="iota")
            nc.sync.dma_start(out=iota_sb[:, :], in_=iota_in[:, :])

            for g in range(GPC):
                psum = psump.tile([K, D + 4], f32, tag="acc")
                tglobal = 0
                for ch in range(chunks_per_group):
                    n0 = g * pad + ch * TPC * 128
                    xt = xpool.tile([128, TPC, D + 4], f32, tag="x")
                    nc.sync.dma_start(
                        out=xt[:, :, 0:D],
                        in_=xg[n0 : n0 + TPC * 128, :].rearrange(
                            "(t p) d -> p t d", p=128
                        ),
                    )
                    nc.sync.dma_start(
                        out=xt[:, :, D : D + 4],
                        in_=ones_in[:, :, :],
                    )
                    xtr = xrpool.tile([128, TPC, D + 4], f32r, tag="xr")
                    nc.scalar.activation(
                        xtr[:, :, :],
                        xt[:, :, :],
                        mybir.ActivationFunctionType.Copy,
                    )
                    blt = blpool.tile([128, TPC], f32, tag="bl")
                    nc.sync.dma_start(
                        out=blt[:, :],
                        in_=bl[g, ch, :, :],
                    )
                    sct = scpool.tile([128, TPC], f32, tag="s")
                    lct = scpool.tile([128, TPC], f32, tag="l")
                    wt = scpool.tile([128, TPC], f32, tag="w")
                    # chunk-batched score dot-product: one DVE mul + one DVE
                    # free-dim reduce for all TPC tiles (per-op overhead on
                    # DVE is ~300ns, so per-tile ops are wasteful; ttr is
                    # broken on this runtime - wedges the device)
                    prod = scrpool.tile([128, TPC, D], f32, tag="prod")
                    nc.gpsimd.tensor_tensor(
                        prod[:, :, :],
                        xt[:, :, 0:D],
                        a_sb[:, :, :],
                        mybir.AluOpType.mult,
                    )
                    nc.vector.tensor_reduce(
                        sct[:, :],
                        prod[:, :, :],
                        mybir.AxisListType.X,
                        mybir.AluOpType.add,
                    )
                    # leaky relu: max(0.2*s, s) on DVE, then exp on ACT
                    nc.vector.tensor_scalar(
                        lct[:, :], sct[:, :], NEG_SLOPE, None, mybir.AluOpType.mult
                    )
                    nc.vector.tensor_tensor(
                        lct[:, :], lct[:, :], sct[:, :], mybir.AluOpType.max
                    )
                    nc.scalar.activation(
                        wt[:, :], lct[:, :], mybir.ActivationFunctionType.Exp
                    )
                    for t in range(TPC):
                        m = mpool.tile([128, K], f32r, tag="m")
                        nc.vector.tensor_scalar(
                            m[:, :],
                            iota_sb[:, :],
                            blt[:, t : t + 1],
                            wt[:, t : t + 1],
                            mybir.AluOpType.is_equal,
                            mybir.AluOpType.mult,
                        )
                        # float32r: same fp32 bits, 1 cyc/row matmul (vs 4 for
                        # plain fp32) when the output AP is >=256 elements
                        nc.tensor.matmul(
                            psum[:, :],
                            m[:, :],
                            xtr[:, t, 0 : D + 4],
                            start=(tglobal == 0),
                            stop=(tglobal == tiles_per_group - 1),
                        )
                        tglobal += 1
                den = eppool.tile([K, 1], f32, tag="den")
                nc.vector.tensor_scalar(
                    den[:, :],
                    psum[:, D : D + 1],
                    1e-16,
                    None,
                    mybir.AluOpType.add,
                )
                rden = eppool.tile([K, 1], f32, tag="rden")
                nc.vector.reciprocal(rden[:, :], den[:, :])
                osb = eppool.tile([K, D], f32, tag="osb")
                nc.vector.tensor_scalar(
                    osb[:, :],
                    psum[:, 0:D],
                    rden[:, :],
                    None,
                    mybir.AluOpType.mult,
                )
                nc.sync.dma_start(out=out[g * K : (g + 1) * K, :], in_=osb[:, :])

    nc.compile()
    return nc


def _prepare_inputs(x, batch, attention_vector):
    """Host-side sharding: group segments, pad each group to a fixed length."""
    x = np.ascontiguousarray(np.asarray(x, dtype=np.float32))
    batch = np.asarray(batch).astype(np.int64)
    a = np.asarray(attention_vector, dtype=np.float32)

    counts = np.bincount(batch, minlength=NSEG)
    offsets = np.zeros(NSEG + 1, np.int64)
    offsets[1:] = np.cumsum(counts)
    gcounts = counts.reshape(-1, K).sum(axis=1)  # [256]
    chunk_nodes = 128 * TPC
    pad = int(np.ceil(gcounts.max() / chunk_nodes) * chunk_nodes)

    cpg = pad // chunk_nodes  # chunks per group
    in_maps = []
    arep = np.broadcast_to(a, (128, TPC, D)).copy()
    iota = np.broadcast_to(np.arange(K, dtype=np.float32), (128, K)).copy()
    ones = np.ones((128, TPC, 4), np.float32)
    for c in range(NCORES):
        xgc = np.zeros((GPC, pad, D), np.float32)
        blc = np.full((GPC, pad), 200.0, np.float32)
        for gi in range(GPC):
            g = c * GPC + gi
            s0 = g * K
            n0, n1 = offsets[s0], offsets[s0 + K]
            L = n1 - n0
            xgc[gi, :L] = x[n0:n1]
            blc[gi, :L] = (batch[n0:n1] - s0).astype(np.float32)
        # [GPC, pad] -> [GPC, cpg, TPC, 128] -> transpose to [GPC, cpg, 128, TPC]
        blc = np.ascontiguousarray(
            blc.reshape(GPC, cpg, TPC, 128).transpose(0, 1, 3, 2)
        )
        in_maps.append(
            {
                "xg": xgc.reshape(GPC * pad, D),
                "bl": blc,
                "arep": arep,
                "iota_in": iota,
                "ones_in": ones,
            }
        )
    return in_maps, pad


_last_results = None


def kernel(x, batch, attention_vector):
    global _last_results
    from concourse.bass_utils import run_bass_kernel_spmd

    in_maps, pad = _prepare_inputs(x, batch, attention_vector)
    if pad not in _prog_cache:
        _prog_cache[pad] = _build_program(pad)
    nc = _prog_cache[pad]
    res = run_bass_kernel_spmd(nc, in_maps, list(range(NCORES)))
    _last_results = res
    outs = [res.results[c]["out"] for c in range(NCORES)]
    return np.concatenate(outs, axis=0).astype(np.float32)
